# revision 10
# baseline (speedup 1.0000x reference)
"""Trainium2 Bass kernel for nn_DDoSDetectionModel (Mamba stack with L=1).

Key simplifications (exact, verified vs reference in fp64/fp32):
  * Sequence length is 1, so the SSM scan is a single step with h0=0:
    A_log never affects the output, and y = delta*x*(Bm.Cm) + D*x.
  * The causal depthwise conv (K=16, left pad 15) on L=1 reduces to its
    last tap: conv(x) = x * conv_w[:, -1] + conv_b.
  * rmsnorm's norm_w and the conv last-tap scale fold into W_in (host-side).

Layout: feature-major on chip — activations stored as [features, batch],
batch on the free dimension (512 rows per core).  The 512 batch columns are
processed as two pipeline blocks of 256 so the Tile scheduler can overlap
one block's matmul phases with the other block's ACT/DVE phases.  Inside
every [128, n*512] activation tile, column  c*512 + blk*256 + b  holds
chunk c, block blk, batch b; ACT/DVE ops run block-merged on [128, 512]
(or bigger) slices while matmuls run per block (N=256).

Sharding: pure data parallel, batch 4096 split across 8 cores.
"""

import numpy as np
import ml_dtypes

D_MODEL = 256
D_STATE = 32
N_LAYERS = 4
D_INNER = 1024
DT_RANK = 16
INPUT_DIM = 78
BATCH = 4096
EPS = 1e-5
NCORES = 8
B = BATCH // NCORES          # 512 batch rows per core
NBLK = 2
BB = B // NBLK               # 256 batch rows per pipeline block
KC_DM = D_MODEL // 128       # 2 k-chunks over d_model
MC_ED = D_INNER // 128       # 8 m-chunks over d_inner
NDBC = 96                    # wx padded: r@0:16, Bm@32:64, Cm@64:96

_CACHE = {}

bf16 = ml_dtypes.bfloat16


def _build_nc():
    import concourse.tile as tile
    from concourse import bacc, mybir

    BF = mybir.dt.bfloat16
    F32 = mybir.dt.float32
    AF = mybir.ActivationFunctionType
    OP = mybir.AluOpType

    nc = bacc.Bacc("TRN2", target_bir_lowering=False, debug=False,
                   num_devices=NCORES)

    # ---- DRAM I/O ----
    d_xT = nc.dram_tensor("xT", [INPUT_DIM, B], BF, kind="ExternalInput").ap()
    d_wp = nc.dram_tensor("wp", [INPUT_DIM, D_MODEL], BF, kind="ExternalInput").ap()
    d_bp = nc.dram_tensor("bp", [128, KC_DM], F32, kind="ExternalInput").ap()
    d_win = nc.dram_tensor("win", [N_LAYERS, D_MODEL, 2 * D_INNER], BF, kind="ExternalInput").ap()
    d_cb = nc.dram_tensor("cb", [N_LAYERS, 128, MC_ED], F32, kind="ExternalInput").ap()
    d_wx = nc.dram_tensor("wx", [N_LAYERS, D_INNER, NDBC], BF, kind="ExternalInput").ap()
    d_wdt = nc.dram_tensor("wdt", [N_LAYERS, DT_RANK, D_INNER], BF, kind="ExternalInput").ap()
    d_bdt = nc.dram_tensor("bdt", [N_LAYERS, 128, MC_ED], F32, kind="ExternalInput").ap()
    d_dd = nc.dram_tensor("dd", [N_LAYERS, 128, MC_ED], F32, kind="ExternalInput").ap()
    d_wout = nc.dram_tensor("wout", [N_LAYERS, D_INNER, D_MODEL], BF, kind="ExternalInput").ap()
    d_wfin = nc.dram_tensor("wfin", [128, KC_DM], BF, kind="ExternalInput").ap()
    d_bfin = nc.dram_tensor("bfin", [1, 1], F32, kind="ExternalInput").ap()
    d_eye = nc.dram_tensor("eye", [128, 128], BF, kind="ExternalInput").ap()
    d_out = nc.dram_tensor("out", [1, B], F32, kind="ExternalOutput").ap()

    with tile.TileContext(nc) as tc, \
         tc.tile_pool(name="const", bufs=1) as constp, \
         tc.tile_pool(name="wbig", bufs=2) as wbig, \
         tc.tile_pool(name="wsmall", bufs=2) as wsmall, \
         tc.tile_pool(name="bias", bufs=2) as biasp, \
         tc.tile_pool(name="act", bufs=2) as actp, \
         tc.tile_pool(name="ed", bufs=2) as edp, \
         tc.tile_pool(name="ebuf", bufs=1) as ebufp, \
         tc.tile_pool(name="small", bufs=2) as smallp, \
         tc.tile_pool(name="mm", bufs=3, space="PSUM") as mmp, \
         tc.tile_pool(name="red", bufs=2, space="PSUM") as redp, \
         tc.tile_pool(name="bc", bufs=1, space="PSUM") as bcp, \
         tc.tile_pool(name="outp", bufs=2, space="PSUM") as outp:

        # ---- constants ----
        ones_col = constp.tile([128, 1], BF, tag="ones_col")
        nc.vector.memset(ones_col[:], 1.0)
        eps_sb = constp.tile([1, 1], F32, tag="eps")
        nc.vector.memset(eps_sb[:], EPS)
        ones_row = constp.tile([1, 128], BF, tag="ones_row")
        nc.vector.memset(ones_row[:], 1.0)
        eye = constp.tile([128, 128], BF, tag="eye")
        nc.sync.dma_start(eye[:], d_eye[:])
        wp_sb = constp.tile([INPUT_DIM, D_MODEL], BF, tag="wp")
        nc.sync.dma_start(wp_sb[:], d_wp[:])
        bp_sb = constp.tile([128, KC_DM], F32, tag="bp")
        nc.sync.dma_start(bp_sb[:], d_bp[:])
        wfin_sb = constp.tile([128, KC_DM], BF, tag="wfin")
        nc.sync.dma_start(wfin_sb[:], d_wfin[:])
        bfin_sb = constp.tile([1, 1], F32, tag="bfin")
        nc.sync.dma_start(bfin_sb[:], d_bfin[:])
        xT_sb = constp.tile([INPUT_DIM, B], BF, tag="xT")
        nc.sync.dma_start(xT_sb[:], d_xT[:])

        # ---- input projection: h = x @ Wp + bp  (feature-major, per block) --
        # h_blk tile layout: [128, KC_DM*BB], chunk-major
        h_sb = [actp.tile([128, KC_DM * BB], BF, tag=f"h{blk}",
                          name=f"h_init{blk}")
                for blk in range(NBLK)]
        for blk in range(NBLK):
            ps = mmp.tile([128, B], F32, tag="mm")
            for kc in range(KC_DM):
                nc.tensor.matmul(ps[:, kc * BB:(kc + 1) * BB],
                                 wp_sb[:, kc * 128:(kc + 1) * 128],
                                 xT_sb[:, blk * BB:(blk + 1) * BB],
                                 start=True, stop=True)
            # bias differs per chunk: two Identity ops
            for kc in range(KC_DM):
                nc.scalar.activation(h_sb[blk][:, kc * BB:(kc + 1) * BB],
                                     ps[:, kc * BB:(kc + 1) * BB],
                                     AF.Identity, bias=bp_sb[:, kc:kc + 1])

        # ---- layers ----
        for l in range(N_LAYERS):
            # -- weights for this layer (streamed; double buffered pools) --
            win_sb = wbig.tile([128, KC_DM * 2 * D_INNER], BF, tag="win")
            nc.sync.dma_start(
                win_sb[:].rearrange("p (kc m) -> p kc m", kc=KC_DM),
                d_win[l].rearrange("(kc p) m -> p kc m", p=128))
            wout_sb = wbig.tile([128, MC_ED * D_MODEL], BF, tag="wout")
            nc.sync.dma_start(
                wout_sb[:].rearrange("p (kc m) -> p kc m", kc=MC_ED),
                d_wout[l].rearrange("(kc p) m -> p kc m", p=128))
            wx_sb = wsmall.tile([128, MC_ED * NDBC], BF, tag="wx")
            nc.sync.dma_start(
                wx_sb[:].rearrange("p (kc m) -> p kc m", kc=MC_ED),
                d_wx[l].rearrange("(kc p) m -> p kc m", p=128))
            wdt_sb = wsmall.tile([DT_RANK, D_INNER], BF, tag="wdt")
            nc.sync.dma_start(wdt_sb[:], d_wdt[l][:])
            cb_sb = biasp.tile([128, MC_ED], F32, tag="cb")
            nc.sync.dma_start(cb_sb[:], d_cb[l][:])
            bdt_sb = biasp.tile([128, MC_ED], F32, tag="bdt")
            nc.sync.dma_start(bdt_sb[:], d_bdt[l][:])
            dd_sb = biasp.tile([128, MC_ED], F32, tag="dd")
            nc.sync.dma_start(dd_sb[:], d_dd[l][:])

            # -- rmsnorm (per block): rstd = exp(-0.5*ln(mean(h^2)+eps)) --
            rstd_bc = bcp.tile([128, B], F32, tag="bc")  # blk halves
            xn_sb = []
            for blk in range(NBLK):
                sq = smallp.tile([128, KC_DM * BB], BF, tag=f"sq{blk}")
                nc.vector.tensor_tensor(sq[:], h_sb[blk][:], h_sb[blk][:],
                                        OP.mult)
                ssq = redp.tile([128, BB], F32, tag="red")
                for kc in range(KC_DM):
                    nc.tensor.matmul(ssq[0:1, :], ones_col[:],
                                     sq[:, kc * BB:(kc + 1) * BB],
                                     start=(kc == 0), stop=(kc == KC_DM - 1))
                lnms = smallp.tile([1, BB], F32, tag=f"lnms{blk}")
                nc.scalar.activation(lnms[:], ssq[0:1, :], AF.Ln,
                                     scale=1.0 / D_MODEL,
                                     bias=eps_sb[0:1, 0:1])
                rstd_row = smallp.tile([1, BB], BF, tag=f"rstd{blk}")
                nc.scalar.activation(rstd_row[:], lnms[:], AF.Exp, scale=-0.5)
                nc.tensor.matmul(rstd_bc[:, blk * BB:(blk + 1) * BB],
                                 ones_row[:], rstd_row[:],
                                 start=True, stop=True)
                xn = smallp.tile([128, KC_DM * BB], BF, tag=f"xn{blk}")
                for kc in range(KC_DM):
                    nc.vector.tensor_tensor(
                        xn[:, kc * BB:(kc + 1) * BB],
                        h_sb[blk][:, kc * BB:(kc + 1) * BB],
                        rstd_bc[:, blk * BB:(blk + 1) * BB], OP.mult)
                xn_sb.append(xn)

            # -- W_in matmuls (per block, N=256) + block-merged silu --
            # ED-wide tiles: col = mc*512 + blk*256 + b
            xi_sb = edp.tile([128, MC_ED * B], BF, tag="xi")
            sz_sb = edp.tile([128, MC_ED * B], BF, tag="sz")
            for mc in range(2 * MC_ED):
                ps = mmp.tile([128, B], F32, tag="mm")
                for blk in range(NBLK):
                    for kc in range(KC_DM):
                        nc.tensor.matmul(
                            ps[:, blk * BB:(blk + 1) * BB],
                            win_sb[:, kc * 2 * D_INNER + mc * 128:
                                   kc * 2 * D_INNER + (mc + 1) * 128],
                            xn_sb[blk][:, kc * BB:(kc + 1) * BB],
                            start=(kc == 0), stop=(kc == KC_DM - 1))
                if mc < MC_ED:
                    nc.scalar.activation(xi_sb[:, mc * B:(mc + 1) * B], ps[:],
                                         AF.Silu, bias=cb_sb[:, mc:mc + 1])
                else:
                    m2 = mc - MC_ED
                    nc.scalar.activation(sz_sb[:, m2 * B:(m2 + 1) * B], ps[:],
                                         AF.Silu)

            # g = xi * silu(z)   (one big DVE op)
            g_sb = edp.tile([128, MC_ED * B], BF, tag="g")
            nc.vector.tensor_tensor(g_sb[:], xi_sb[:], sz_sb[:], OP.mult)

            # -- dbc = xi @ Wx (per block) + s = sum(Bm*Cm) --
            r_sb, sbc_list = [], []
            sbc_ps = bcp.tile([128, B], F32, tag="bc")  # s broadcast halves
            for blk in range(NBLK):
                dbc_ps = redp.tile([128, BB], F32, tag="red")
                for kc in range(MC_ED):
                    nc.tensor.matmul(
                        dbc_ps[0:NDBC, :],
                        wx_sb[:, kc * NDBC:(kc + 1) * NDBC],
                        xi_sb[:, kc * B + blk * BB:kc * B + (blk + 1) * BB],
                        start=(kc == 0), stop=(kc == MC_ED - 1))
                r = smallp.tile([DT_RANK, BB], BF, tag=f"r{blk}")
                nc.vector.tensor_copy(r[:], dbc_ps[0:DT_RANK, :])
                r_sb.append(r)
                bm = smallp.tile([D_STATE, BB], BF, tag=f"bm{blk}")
                nc.vector.tensor_copy(bm[:], dbc_ps[32:64, :])
                cm = smallp.tile([D_STATE, BB], BF, tag=f"cm{blk}")
                nc.vector.tensor_copy(cm[:], dbc_ps[64:96, :])
                bmcm = smallp.tile([D_STATE, BB], BF, tag=f"bmcm{blk}")
                nc.vector.tensor_tensor(bmcm[:], bm[:], cm[:], OP.mult)
                s_ps = redp.tile([128, BB], F32, tag="red")
                nc.tensor.matmul(s_ps[0:1, :], ones_col[0:D_STATE, :],
                                 bmcm[:], start=True, stop=True)
                s_row = smallp.tile([1, BB], BF, tag=f"s_row{blk}")
                nc.vector.tensor_copy(s_row[:], s_ps[0:1, :])
                nc.tensor.matmul(sbc_ps[:, blk * BB:(blk + 1) * BB],
                                 ones_row[:], s_row[:], start=True, stop=True)
            s_bc = smallp.tile([128, B], BF, tag="s_bc")
            nc.vector.tensor_copy(s_bc[:], sbc_ps[:])

            # -- delta = softplus(r @ Wdt + bdt) = ln(1 + exp(.)) --
            e_sb = ebufp.tile([128, MC_ED * B], F32, tag="e")
            for mc in range(MC_ED):
                dt_ps = mmp.tile([128, B], F32, tag="mm")
                for blk in range(NBLK):
                    nc.tensor.matmul(dt_ps[:, blk * BB:(blk + 1) * BB],
                                     wdt_sb[:, mc * 128:(mc + 1) * 128],
                                     r_sb[blk][:], start=True, stop=True)
                nc.scalar.activation(e_sb[:, mc * B:(mc + 1) * B], dt_ps[:],
                                     AF.Exp, bias=bdt_sb[:, mc:mc + 1])
            delta_sb = edp.tile([128, MC_ED * B], BF, tag="delta")
            nc.scalar.activation(delta_sb[:], e_sb[:], AF.Ln, bias=1.0)

            # -- pre = (delta*s + D) * g --
            t1_sb = edp.tile([128, MC_ED * B], BF, tag="t1")
            nc.vector.tensor_tensor(
                t1_sb[:].rearrange("p (c b) -> p c b", c=MC_ED),
                delta_sb[:].rearrange("p (c b) -> p c b", c=MC_ED),
                s_bc[:].unsqueeze(1).broadcast_to((128, MC_ED, B)), OP.mult)
            u_sb = edp.tile([128, MC_ED * B], BF, tag="u")
            for mc in range(MC_ED):
                nc.vector.tensor_scalar(u_sb[:, mc * B:(mc + 1) * B],
                                        t1_sb[:, mc * B:(mc + 1) * B],
                                        dd_sb[:, mc:mc + 1], None, OP.add)
            pre_sb = edp.tile([128, MC_ED * B], BF, tag="pre")
            nc.vector.tensor_tensor(pre_sb[:], u_sb[:], g_sb[:], OP.mult)

            # -- h = h + pre @ W_out  (residual via identity matmul) --
            hn_sb = [actp.tile([128, KC_DM * BB], BF, tag=f"h{blk}",
                              name=f"h_l{l}_{blk}")
                     for blk in range(NBLK)]
            for blk in range(NBLK):
                ops = outp.tile([128, KC_DM * BB], F32, tag="outp")
                for mc in range(KC_DM):
                    for kc in range(MC_ED):
                        nc.tensor.matmul(
                            ops[:, mc * BB:(mc + 1) * BB],
                            wout_sb[:, kc * D_MODEL + mc * 128:
                                    kc * D_MODEL + (mc + 1) * 128],
                            pre_sb[:, kc * B + blk * BB:
                                   kc * B + (blk + 1) * BB],
                            start=(kc == 0), stop=False)
                    nc.tensor.matmul(ops[:, mc * BB:(mc + 1) * BB], eye[:],
                                     h_sb[blk][:, mc * BB:(mc + 1) * BB],
                                     start=False, stop=True)
                nc.vector.tensor_copy(hn_sb[blk][:], ops[:])
            h_sb = hn_sb

        # ---- head: sigmoid(h @ W_final + b_final) via tanh ----
        for blk in range(NBLK):
            fin_ps = redp.tile([128, BB], F32, tag="red")
            for kc in range(KC_DM):
                nc.tensor.matmul(fin_ps[0:1, :], wfin_sb[:, kc:kc + 1],
                                 h_sb[blk][:, kc * BB:(kc + 1) * BB],
                                 start=(kc == 0), stop=(kc == KC_DM - 1))
            th = smallp.tile([1, BB], F32, tag=f"th{blk}")
            nc.scalar.activation(th[:], fin_ps[0:1, :], AF.Tanh,
                                 scale=0.5, bias=bfin_sb[0:1, 0:1])
            orow = smallp.tile([1, BB], F32, tag=f"orow{blk}")
            nc.vector.tensor_scalar(orow[:], th[:], 0.5, 0.5, OP.mult, OP.add)
            nc.sync.dma_start(d_out[:, blk * BB:(blk + 1) * BB], orow[:])

    nc.compile()
    return nc


def _prep_inputs(inputs):
    """Host-side weight preprocessing (dtype casts, folds, layouts)."""
    f = {k: np.asarray(v, dtype=np.float32) for k, v in inputs.items()}

    win_eff = f["W_in"] * f["norm_w"][:, :, None]          # fold rmsnorm gain
    win_eff[:, :, :D_INNER] *= f["conv_w"][:, None, :, -1]  # fold conv last tap

    def chunk_cols(v):  # [L, 1024] -> [L, 128, 8] (partition-major per chunk)
        return np.ascontiguousarray(
            v.reshape(N_LAYERS, MC_ED, 128).transpose(0, 2, 1))

    com = {
        "wp": f["W_proj_in"].astype(bf16),
        "bp": np.ascontiguousarray(
            f["b_proj_in"].reshape(KC_DM, 128).T).astype(np.float32),
        "win": win_eff.astype(bf16),
        "cb": chunk_cols(f["conv_b"]).astype(np.float32),
        "wx": np.concatenate([
            f["W_x"][:, :, :DT_RANK],
            np.zeros((N_LAYERS, D_INNER, 16), np.float32),
            f["W_x"][:, :, DT_RANK:],
        ], axis=2).astype(bf16),
        "wdt": f["W_dt"].astype(bf16),
        "bdt": chunk_cols(f["b_dt"]).astype(np.float32),
        "dd": chunk_cols(f["D"]).astype(np.float32),
        "wout": f["W_out"].astype(bf16),
        "wfin": np.ascontiguousarray(
            f["W_final"].reshape(KC_DM, 128).T).astype(bf16),
        "bfin": (0.5 * f["b_final"]).reshape(1, 1).astype(np.float32),
        "eye": np.eye(128, dtype=np.float32).astype(bf16),
    }
    shards = []
    x = f["x"]
    for c in range(NCORES):
        xs = x[c * B:(c + 1) * B]                      # [512, 78]
        m = dict(com)
        m["xT"] = np.ascontiguousarray(xs.T).astype(bf16)
        shards.append(m)
    return shards


def kernel(**inputs):
    from concourse.bass_utils import run_bass_kernel_spmd

    if "nc" not in _CACHE:
        _CACHE["nc"] = _build_nc()
    nc = _CACHE["nc"]

    in_maps = _prep_inputs(inputs)
    res = run_bass_kernel_spmd(nc, in_maps, core_ids=list(range(NCORES)))
    out = np.concatenate(
        [res.results[c]["out"].reshape(B, 1) for c in range(NCORES)], axis=0)
    return out.astype(np.float32)


if __name__ == "__main__":
    nc = _build_nc()
    print("build+compile OK")


# revision 13
# speedup vs baseline: 1.0141x; 1.0141x over previous
"""Trainium2 Bass kernel for nn_DDoSDetectionModel (Mamba stack with L=1).

Key simplifications (exact, verified vs reference in fp64/fp32):
  * Sequence length is 1, so the SSM scan is a single step with h0=0:
    A_log never affects the output, and y = delta*x*(Bm.Cm) + D*x.
  * The causal depthwise conv (K=16, left pad 15) on L=1 reduces to its
    last tap: conv(x) = x * conv_w[:, -1] + conv_b.
  * rmsnorm's norm_w and the conv last-tap scale fold into W_in (host-side).
  * b_proj / b_dt biases ride the matmuls as an extra contraction row
    (K=78->79 and K=16->17) against a constant ones row.

Layout: feature-major on chip — activations stored as [features, batch],
batch (512 rows/core) on the free dimension, so every linear layer is
matmul(out=y_T, lhsT=W, rhs=x_T) with W in its natural [in, out] layout.

Sharding: pure data parallel, batch 4096 split across 8 cores.
"""

import numpy as np
import ml_dtypes

D_MODEL = 256
D_STATE = 32
N_LAYERS = 4
D_INNER = 1024
DT_RANK = 16
INPUT_DIM = 78
BATCH = 4096
EPS = 1e-5
NCORES = 8
B = BATCH // NCORES          # 512 batch rows per core
KC_DM = D_MODEL // 128       # 2 k-chunks over d_model
MC_ED = D_INNER // 128       # 8 m-chunks over d_inner
NDBC = 96                    # wx padded: r@0:16, Bm@32:64, Cm@64:96

_CACHE = {}

bf16 = ml_dtypes.bfloat16


def _build_nc():
    import concourse.tile as tile
    from concourse import bacc, mybir

    BF = mybir.dt.bfloat16
    F32 = mybir.dt.float32
    AF = mybir.ActivationFunctionType
    OP = mybir.AluOpType

    nc = bacc.Bacc("TRN2", target_bir_lowering=False, debug=False,
                   num_devices=NCORES)

    # ---- DRAM I/O ----
    d_xT = nc.dram_tensor("xT", [INPUT_DIM + 1, B], BF, kind="ExternalInput").ap()
    d_wp = nc.dram_tensor("wp", [INPUT_DIM + 1, D_MODEL], BF, kind="ExternalInput").ap()
    d_win = nc.dram_tensor("win", [N_LAYERS, D_MODEL, 2 * D_INNER], BF, kind="ExternalInput").ap()
    d_cb = nc.dram_tensor("cb", [N_LAYERS, 128, MC_ED], F32, kind="ExternalInput").ap()
    d_wx = nc.dram_tensor("wx", [N_LAYERS, D_INNER, NDBC], BF, kind="ExternalInput").ap()
    d_wdt = nc.dram_tensor("wdt", [N_LAYERS, DT_RANK, D_INNER], BF, kind="ExternalInput").ap()
    d_bdt = nc.dram_tensor("bdt", [N_LAYERS, 128, MC_ED], F32, kind="ExternalInput").ap()
    d_dd = nc.dram_tensor("dd", [N_LAYERS, 128, MC_ED], F32, kind="ExternalInput").ap()
    d_wout = nc.dram_tensor("wout", [N_LAYERS, D_INNER, D_MODEL], BF, kind="ExternalInput").ap()
    d_wfin = nc.dram_tensor("wfin", [128, KC_DM], BF, kind="ExternalInput").ap()
    d_bfin = nc.dram_tensor("bfin", [1, 1], F32, kind="ExternalInput").ap()
    d_eye = nc.dram_tensor("eye", [128, 128], BF, kind="ExternalInput").ap()
    d_out = nc.dram_tensor("out", [1, B], F32, kind="ExternalOutput").ap()

    with tile.TileContext(nc) as tc, \
         tc.tile_pool(name="const", bufs=1) as constp, \
         tc.tile_pool(name="wbig", bufs=2) as wbig, \
         tc.tile_pool(name="wsmall", bufs=2) as wsmall, \
         tc.tile_pool(name="bias", bufs=2) as biasp, \
         tc.tile_pool(name="act", bufs=2) as actp, \
         tc.tile_pool(name="ed", bufs=2) as edp, \
         tc.tile_pool(name="ebuf", bufs=2) as ebufp, \
         tc.tile_pool(name="small", bufs=2) as smallp, \
         tc.tile_pool(name="mm", bufs=4, space="PSUM") as mmp, \
         tc.tile_pool(name="red", bufs=2, space="PSUM") as redp, \
         tc.tile_pool(name="outp", bufs=2, space="PSUM") as outp:

        # ---- constants ----
        ones_col = constp.tile([128, 1], BF, tag="ones_col")
        nc.vector.memset(ones_col[:], 1.0)
        eps_sb = constp.tile([1, 1], F32, tag="eps")
        nc.vector.memset(eps_sb[:], EPS)
        ones_row = constp.tile([1, 128], BF, tag="ones_row")
        nc.vector.memset(ones_row[:], 1.0)
        ones_b = constp.tile([1, B], BF, tag="ones_b")
        nc.vector.memset(ones_b[:], 1.0)
        eye = constp.tile([128, 128], BF, tag="eye")
        nc.sync.dma_start(eye[:], d_eye[:])
        wp_sb = constp.tile([INPUT_DIM + 1, D_MODEL], BF, tag="wp")
        nc.sync.dma_start(wp_sb[:], d_wp[:])
        wfin_sb = constp.tile([128, KC_DM], BF, tag="wfin")
        nc.sync.dma_start(wfin_sb[:], d_wfin[:])
        bfin_sb = constp.tile([1, 1], F32, tag="bfin")
        nc.sync.dma_start(bfin_sb[:], d_bfin[:])
        xT_sb = constp.tile([INPUT_DIM + 1, B], BF, tag="xT")
        nc.sync.dma_start(xT_sb[:], d_xT[:])

        # ---- input projection: h = x_aug @ Wp_aug  (bias via ones row) ----
        h_sb = actp.tile([128, KC_DM * B], BF, tag="h", name="h_init")
        for kc in range(KC_DM):
            hp = mmp.tile([128, B], F32, tag="mm", name=f"hp{kc}")
            nc.tensor.matmul(hp[:], wp_sb[:, kc * 128:(kc + 1) * 128],
                             xT_sb[:], start=True, stop=True)
            nc.vector.tensor_copy(h_sb[:, kc * B:(kc + 1) * B], hp[:])

        # ---- layers ----
        for l in range(N_LAYERS):
            # -- weights for this layer (streamed; double buffered pools) --
            win_sb = wbig.tile([128, KC_DM * 2 * D_INNER], BF, tag="win")
            nc.sync.dma_start(
                win_sb[:].rearrange("p (kc m) -> p kc m", kc=KC_DM),
                d_win[l].rearrange("(kc p) m -> p kc m", p=128))
            wout_sb = wbig.tile([128, MC_ED * D_MODEL], BF, tag="wout")
            nc.sync.dma_start(
                wout_sb[:].rearrange("p (kc m) -> p kc m", kc=MC_ED),
                d_wout[l].rearrange("(kc p) m -> p kc m", p=128))
            wx_sb = wsmall.tile([128, MC_ED * NDBC], BF, tag="wx")
            nc.sync.dma_start(
                wx_sb[:].rearrange("p (kc m) -> p kc m", kc=MC_ED),
                d_wx[l].rearrange("(kc p) m -> p kc m", p=128))
            wdt_sb = wsmall.tile([DT_RANK, D_INNER], BF, tag="wdt")
            nc.sync.dma_start(wdt_sb[:], d_wdt[l][:])
            bdt_sb = biasp.tile([128, MC_ED], F32, tag="bdt")
            nc.sync.dma_start(bdt_sb[:], d_bdt[l][:])
            cb_sb = biasp.tile([128, MC_ED], F32, tag="cb")
            nc.sync.dma_start(cb_sb[:], d_cb[l][:])
            dd_sb = biasp.tile([128, MC_ED], F32, tag="dd")
            nc.sync.dma_start(dd_sb[:], d_dd[l][:])

            # -- rmsnorm: rstd = exp(-0.5*ln(mean(h^2)+eps)) --
            sq_sb = smallp.tile([128, KC_DM * B], BF, tag="sq")
            ssq = redp.tile([128, B], F32, tag="red")
            for kc in range(KC_DM):
                nc.vector.tensor_tensor(
                    sq_sb[:, kc * B:(kc + 1) * B],
                    h_sb[:, kc * B:(kc + 1) * B],
                    h_sb[:, kc * B:(kc + 1) * B], OP.mult)
                nc.tensor.matmul(ssq[0:1, :], ones_col[:],
                                 sq_sb[:, kc * B:(kc + 1) * B],
                                 start=(kc == 0), stop=(kc == KC_DM - 1))
            lnms = smallp.tile([1, B], F32, tag="lnms")
            nc.scalar.activation(lnms[:], ssq[0:1, :], AF.Ln,
                                 scale=1.0 / D_MODEL, bias=eps_sb[0:1, 0:1])
            rstd_row = smallp.tile([1, B], BF, tag="rstd_row")
            nc.scalar.activation(rstd_row[:], lnms[:], AF.Exp, scale=-0.5)
            rstd_ps = redp.tile([128, B], F32, tag="red", name="rstd_ps")
            nc.tensor.matmul(rstd_ps[:], ones_row[:], rstd_row[:],
                             start=True, stop=True)
            xn_sb = smallp.tile([128, KC_DM * B], BF, tag="xn")
            for kc in range(KC_DM):
                nc.vector.tensor_tensor(
                    xn_sb[:, kc * B:(kc + 1) * B],
                    h_sb[:, kc * B:(kc + 1) * B], rstd_ps[:], OP.mult)

            # -- W_in matmuls: xi chunks first (z deferred to fill gaps) --
            xi_sb = edp.tile([128, MC_ED * B], BF, tag="xi")
            sz_sb = edp.tile([128, MC_ED * B], BF, tag="sz")
            for mc in range(MC_ED):
                ps = mmp.tile([128, B], F32, tag="mm")
                for kc in range(KC_DM):
                    nc.tensor.matmul(
                        ps[:],
                        win_sb[:, kc * 2 * D_INNER + mc * 128:
                               kc * 2 * D_INNER + (mc + 1) * 128],
                        xn_sb[:, kc * B:(kc + 1) * B],
                        start=(kc == 0), stop=(kc == KC_DM - 1))
                nc.scalar.activation(xi_sb[:, mc * B:(mc + 1) * B], ps[:],
                                     AF.Silu, bias=cb_sb[:, mc:mc + 1])

            # -- dbc = xi @ Wx --
            dbc_ps = redp.tile([128, B], F32, tag="red")
            for kc in range(MC_ED):
                nc.tensor.matmul(
                    dbc_ps[0:NDBC, :],
                    wx_sb[:, kc * NDBC:(kc + 1) * NDBC],
                    xi_sb[:, kc * B:(kc + 1) * B],
                    start=(kc == 0), stop=(kc == MC_ED - 1))

            # -- z matmuls + silu (fills the ACT gap of the dbc phase) --
            for m2 in range(MC_ED):
                mc = MC_ED + m2
                ps = mmp.tile([128, B], F32, tag="mm", name=f"zps{m2}")
                for kc in range(KC_DM):
                    nc.tensor.matmul(
                        ps[:],
                        win_sb[:, kc * 2 * D_INNER + mc * 128:
                               kc * 2 * D_INNER + (mc + 1) * 128],
                        xn_sb[:, kc * B:(kc + 1) * B],
                        start=(kc == 0), stop=(kc == KC_DM - 1))
                nc.scalar.activation(sz_sb[:, m2 * B:(m2 + 1) * B],
                                     ps[:], AF.Silu)

            # r/Bm/Cm out of psum (32-aligned bases); s = sum(Bm*Cm)
            r_sb = smallp.tile([DT_RANK, B], BF, tag="r_sb")
            nc.vector.tensor_copy(r_sb[:], dbc_ps[0:DT_RANK, :])
            bm_sb = smallp.tile([D_STATE, B], BF, tag="bm_sb")
            nc.vector.tensor_copy(bm_sb[:], dbc_ps[32:64, :])
            cm_sb = smallp.tile([D_STATE, B], BF, tag="cm_sb")
            nc.vector.tensor_copy(cm_sb[:], dbc_ps[64:96, :])
            bmcm = smallp.tile([D_STATE, B], BF, tag="bmcm")
            nc.vector.tensor_tensor(bmcm[:], bm_sb[:], cm_sb[:], OP.mult)
            s_ps = redp.tile([128, B], F32, tag="red")
            nc.tensor.matmul(s_ps[0:1, :], ones_col[0:D_STATE, :], bmcm[:],
                             start=True, stop=True)
            s_row = smallp.tile([1, B], BF, tag="s_row")
            nc.vector.tensor_copy(s_row[:], s_ps[0:1, :])
            sbc_ps = redp.tile([128, B], F32, tag="red", name="sbc_ps")
            nc.tensor.matmul(sbc_ps[:], ones_row[:], s_row[:],
                             start=True, stop=True)
            s_bc = smallp.tile([128, B], BF, tag="s_bc")
            nc.vector.tensor_copy(s_bc[:], sbc_ps[:])

            # -- delta path, pipelined per chunk-pair:
            #    dt (bias via ones row) -> exp -> ln -> t1 -> u -> pre --
            g_sb = edp.tile([128, MC_ED * B], BF, tag="g")
            pre_sb = edp.tile([128, MC_ED * B], BF, tag="pre")
            e_sb = ebufp.tile([128, 2 * B], F32, tag="e")
            delta_sb = ebufp.tile([128, 2 * B], BF, tag="delta")
            t1_sb = ebufp.tile([128, 2 * B], BF, tag="t1")
            u_sb = ebufp.tile([128, 2 * B], BF, tag="u")
            for mp in range(MC_ED // 2):
                lo, hi = 2 * mp * B, (2 * mp + 2) * B
                # g for this pair (DVE) — xi*silu(z)
                nc.vector.tensor_tensor(g_sb[:, lo:hi], xi_sb[:, lo:hi],
                                        sz_sb[:, lo:hi], OP.mult)
                e_sb = ebufp.tile([128, 2 * B], F32, tag="e",
                                  name=f"e_{l}_{mp}")
                for half in range(2):
                    mc = 2 * mp + half
                    dt_ps = mmp.tile([128, B], F32, tag="mm",
                                     name=f"dtps{mp}_{half}")
                    nc.tensor.matmul(dt_ps[:],
                                     wdt_sb[:, mc * 128:(mc + 1) * 128],
                                     r_sb[:], start=True, stop=True)
                    nc.scalar.activation(e_sb[:, half * B:(half + 1) * B],
                                         dt_ps[:], AF.Exp,
                                         bias=bdt_sb[:, mc:mc + 1])
                delta_sb = ebufp.tile([128, 2 * B], BF, tag="delta",
                                      name=f"delta_{l}_{mp}")
                nc.scalar.activation(delta_sb[:], e_sb[:], AF.Ln, bias=1.0)
                t1_sb = ebufp.tile([128, 2 * B], BF, tag="t1",
                                   name=f"t1_{l}_{mp}")
                nc.vector.tensor_tensor(
                    t1_sb[:].rearrange("p (c b) -> p c b", c=2),
                    delta_sb[:].rearrange("p (c b) -> p c b", c=2),
                    s_bc[:].unsqueeze(1).broadcast_to((128, 2, B)), OP.mult)
                u_sb = ebufp.tile([128, 2 * B], BF, tag="u",
                                  name=f"u_{l}_{mp}")
                for half in range(2):
                    mc = 2 * mp + half
                    nc.vector.tensor_scalar(u_sb[:, half * B:(half + 1) * B],
                                            t1_sb[:, half * B:(half + 1) * B],
                                            dd_sb[:, mc:mc + 1], None, OP.add)
                nc.vector.tensor_tensor(pre_sb[:, lo:hi], u_sb[:],
                                        g_sb[:, lo:hi], OP.mult)

            # -- h = h + pre @ W_out  (residual via identity matmul) --
            hn_sb = actp.tile([128, KC_DM * B], BF, tag="h", name=f"h_l{l}")
            for mc in range(KC_DM):
                ops = outp.tile([128, B], F32, tag="outp")
                for kc in range(MC_ED):
                    nc.tensor.matmul(
                        ops[:],
                        wout_sb[:, kc * D_MODEL + mc * 128:
                                kc * D_MODEL + (mc + 1) * 128],
                        pre_sb[:, kc * B:(kc + 1) * B],
                        start=(kc == 0), stop=False)
                nc.tensor.matmul(ops[:], eye[:], h_sb[:, mc * B:(mc + 1) * B],
                                 start=False, stop=True)
                nc.vector.tensor_copy(hn_sb[:, mc * B:(mc + 1) * B], ops[:])
            h_sb = hn_sb

        # ---- head: sigmoid(h @ W_final + b_final) via tanh ----
        fin_ps = redp.tile([128, B], F32, tag="red")
        for kc in range(KC_DM):
            nc.tensor.matmul(fin_ps[0:1, :], wfin_sb[:, kc:kc + 1],
                             h_sb[:, kc * B:(kc + 1) * B],
                             start=(kc == 0), stop=(kc == KC_DM - 1))
        th = smallp.tile([1, B], F32, tag="th")
        nc.scalar.activation(th[:], fin_ps[0:1, :], AF.Tanh,
                             scale=0.5, bias=bfin_sb[0:1, 0:1])
        orow = smallp.tile([1, B], F32, tag="orow")
        nc.vector.tensor_scalar(orow[:], th[:], 0.5, 0.5, OP.mult, OP.add)
        nc.sync.dma_start(d_out[:], orow[:])

    nc.compile()
    return nc


def _prep_inputs(inputs):
    """Host-side weight preprocessing (dtype casts, folds, layouts)."""
    f = {k: np.asarray(v, dtype=np.float32) for k, v in inputs.items()}

    win_eff = f["W_in"] * f["norm_w"][:, :, None]          # fold rmsnorm gain
    win_eff[:, :, :D_INNER] *= f["conv_w"][:, None, :, -1]  # fold conv last tap

    def chunk_cols(v):  # [L, 1024] -> [L, 128, 8] (partition-major per chunk)
        return np.ascontiguousarray(
            v.reshape(N_LAYERS, MC_ED, 128).transpose(0, 2, 1))

    com = {
        "wp": np.concatenate([f["W_proj_in"], f["b_proj_in"][None, :]],
                             axis=0).astype(bf16),
        "win": win_eff.astype(bf16),
        "cb": chunk_cols(f["conv_b"]).astype(np.float32),
        "wx": np.concatenate([
            f["W_x"][:, :, :DT_RANK],
            np.zeros((N_LAYERS, D_INNER, 16), np.float32),
            f["W_x"][:, :, DT_RANK:],
        ], axis=2).astype(bf16),
        "wdt": f["W_dt"].astype(bf16),
        "bdt": chunk_cols(f["b_dt"]).astype(np.float32),
        "dd": chunk_cols(f["D"]).astype(np.float32),
        "wout": f["W_out"].astype(bf16),
        "wfin": np.ascontiguousarray(
            f["W_final"].reshape(KC_DM, 128).T).astype(bf16),
        "bfin": (0.5 * f["b_final"]).reshape(1, 1).astype(np.float32),
        "eye": np.eye(128, dtype=np.float32).astype(bf16),
    }
    shards = []
    x = f["x"]
    ones = np.ones((1, B), np.float32)
    for c in range(NCORES):
        xs = x[c * B:(c + 1) * B]                      # [512, 78]
        m = dict(com)
        m["xT"] = np.concatenate([np.ascontiguousarray(xs.T), ones],
                                 axis=0).astype(bf16)
        shards.append(m)
    return shards


def kernel(**inputs):
    from concourse.bass_utils import run_bass_kernel_spmd

    if "nc" not in _CACHE:
        _CACHE["nc"] = _build_nc()
    nc = _CACHE["nc"]

    in_maps = _prep_inputs(inputs)
    res = run_bass_kernel_spmd(nc, in_maps, core_ids=list(range(NCORES)))
    out = np.concatenate(
        [res.results[c]["out"].reshape(B, 1) for c in range(NCORES)], axis=0)
    return out.astype(np.float32)


if __name__ == "__main__":
    nc = _build_nc()
    print("build+compile OK")
